# revision 19
# baseline (speedup 1.0000x reference)
"""Distributed Trainium2 Bass kernel for nn_Attention_87368224735328.

reference:
    score = einsum("bqd,bkd->bqk", enc_outputs, atten_outputs)   # [B,S1,S2]
    alignment = softmax(score, axis=-1)                          # over S2
    out = einsum("bqk,bqd->bkd", alignment, enc_outputs + enc_residual)

Sharding: 8 cores = (batch b in 0..3) x (S2-half in 0..1). Each core computes
its local [S1, S2/2] score block, local softmax row-stats (max / sum-exp over
its S2 half), exchanges the tiny [S1] stats with its partner core, and runs
the second GEMM fully locally (contraction over S1 is complete on every
core). Output shard: [S2/2, D] -> out[b, half].

Stats exchange: one-chip 8-core AllGather (the 2-rank-group collective path
measures ~16x slower than the 8-core path on this stack), with the partner's
slice extracted rank-agnostically via a host-provided one-hot mask so the
SPMD graph stays identical across cores. The exchange is split in three
(q-tile boundaries SPLITS) so every AllGather's latency hides under
TensorEngine work: the early ones under remaining GEMM1 tiles, the last
under GEMM2's earlier-phase accumulation (8 concurrently-open PSUM groups).

Precision: fp16 operands on the TensorEngine (full rate, ~16x finer mantissa
than bf16 -- needed because the scores have std ~32 so softmax is nearly
one-hot and bf16 score error flips argmaxes). Accumulation is f32 in PSUM,
stats/softmax math in f32. Measured end-to-end rel err vs f32 reference ~1.6e-3.
"""

import numpy as np

from concourse import bacc, mybir, tile
from concourse.bass_utils import run_bass_kernel_spmd

B, S, D = 4, 2048, 1024
S2L = S // 2          # local S2 columns per core
NQT = S // 128        # 16 q tiles (S1)
NDC = D // 128        # 8 contraction chunks for GEMM1
NKB = S2L // 512      # 2 PSUM blocks of 512 for GEMM1
NKT = S2L // 128      # 8 output k tiles for GEMM2
SPLITS = (10, 14)     # stats-exchange boundaries (in q tiles)
FP16 = mybir.dt.float16
F32 = mybir.dt.float32
N_CORES = 8
RG8 = [[0, 1, 2, 3, 4, 5, 6, 7]]


def _emit_stats_exchange(nc, P, DR, sel_sb, negm, zloc, cs, lo, hi, tag,
                         use_collective):
    """AllGather all cores' (-m, z) for q tiles [lo, hi), pick the partner's
    slice with the one-hot mask, and write cs[:, lo:hi]."""
    n = hi - lo
    stats_in = DR.tile([128, 2 * n], F32, name=f"stats_in{tag}")
    stats_out = DR.tile([N_CORES, 128, 2 * n], F32, name=f"stats_out{tag}")
    nc.sync.dma_start(out=stats_in[:, 0:n], in_=negm[:, lo:hi])
    nc.sync.dma_start(out=stats_in[:, n:2 * n], in_=zloc[:, lo:hi])
    if use_collective:
        nc.gpsimd.collective_compute(
            "AllGather", mybir.AluOpType.bypass,
            replica_groups=RG8,
            ins=[stats_in[:, :].opt()],
            outs=[stats_out[:, :, :].opt()],
        )
    else:  # debug/sim variant: pretend every rank has our stats
        for r in range(N_CORES):
            nc.sync.dma_start(out=stats_out[r], in_=stats_in[:, :])
    gath = P.tile([128, N_CORES, 2 * n], F32, tag=f"gath{tag}",
                  name=f"gath{tag}")
    nc.sync.dma_start(out=gath[:, :, :],
                      in_=stats_out[:, :, :].rearrange("r p c -> p r c"))

    # partner slice = sum_r sel[r] * gath[r]  (sel is one-hot at partner)
    acc = P.tile([128, 2 * n], F32, tag=f"acc{tag}", name=f"acc{tag}")
    nc.vector.tensor_scalar_mul(out=acc[:, :], in0=gath[:, 0, :],
                                scalar1=sel_sb[:, 0:1])
    for r in range(1, N_CORES):
        nc.vector.scalar_tensor_tensor(
            out=acc[:, :], in0=gath[:, r, :], scalar=sel_sb[:, r:r + 1],
            in1=acc[:, :], op0=mybir.AluOpType.mult, op1=mybir.AluOpType.add)

    # all in negated-max terms: ng = -m_glob = min(negm0, negm1);
    # t_i = exp(ng - negm_i) = exp(m_i - m_glob)
    n0, z0 = negm[:, lo:hi], zloc[:, lo:hi]
    n1, z1 = acc[:, 0:n], acc[:, n:2 * n]
    ng = P.tile([128, n], F32, tag=f"ng{tag}", name=f"ng{tag}")
    t0 = P.tile([128, n], F32, tag=f"t0{tag}", name=f"t0{tag}")
    t1 = P.tile([128, n], F32, tag=f"t1{tag}", name=f"t1{tag}")
    zg = P.tile([128, n], F32, tag=f"zg{tag}", name=f"zg{tag}")
    rz = P.tile([128, n], F32, tag=f"rz{tag}", name=f"rz{tag}")
    nc.vector.tensor_tensor(out=ng[:, :], in0=n0, in1=n1,
                            op=mybir.AluOpType.min)
    nc.vector.tensor_sub(out=t0[:, :], in0=ng[:, :], in1=n0)
    nc.vector.tensor_sub(out=t1[:, :], in0=ng[:, :], in1=n1)
    nc.scalar.activation(out=t0[:, :], in_=t0[:, :],
                         func=mybir.ActivationFunctionType.Exp)
    nc.scalar.activation(out=t1[:, :], in_=t1[:, :],
                         func=mybir.ActivationFunctionType.Exp)
    nc.vector.tensor_mul(out=zg[:, :], in0=t0[:, :], in1=z0)
    nc.vector.tensor_mul(out=t1[:, :], in0=t1[:, :], in1=z1)
    nc.vector.tensor_add(out=zg[:, :], in0=zg[:, :], in1=t1[:, :])
    # c = exp(m_loc - m_glob) / Z_glob = t0 / Z_glob
    nc.vector.reciprocal(out=rz[:, :], in_=zg[:, :])
    nc.vector.tensor_mul(out=cs[:, lo:hi], in0=t0[:, :], in1=rz[:, :])


def _emit_body(nc, tc, pools, qT, kT, enc, res, sel, out, use_collective):
    P, ST, PS, OST, DR = pools

    # ---- persistent SBUF tensors -------------------------------
    qt_sb = [P.tile([128, S], FP16, tag=f"qt{c}", name=f"qt{c}")
             for c in range(NDC)]
    kt_sb = [P.tile([128, S2L], FP16, tag=f"kt{c}", name=f"kt{c}")
             for c in range(NDC)]
    v_sb = [P.tile([128, D], FP16, tag=f"v{i}", name=f"v{i}")
            for i in range(NQT)]
    e_sb = [P.tile([128, S2L], FP16, tag=f"e{i}", name=f"e{i}")
            for i in range(NQT)]
    negm = P.tile([128, NQT], F32, tag="negm", name="negm")
    zloc = P.tile([128, NQT], F32, tag="zloc", name="zloc")
    cs = P.tile([128, NQT], F32, tag="cs", name="cs")
    sel_sb = P.tile([128, N_CORES], F32, tag="sel", name="sel_sb")

    # ---- load GEMM1 operands (d on partitions, pre-transposed) --
    # first chunk pair split finer so the very first matmuls start sooner
    for c in range(NDC):
        if c == 0:
            nc.sync.dma_start(out=qt_sb[c][:, 0:512],
                              in_=qT[c * 128:(c + 1) * 128, 0:512])
            nc.sync.dma_start(out=kt_sb[c][:, 0:512],
                              in_=kT[c * 128:(c + 1) * 128, 0:512])
            nc.sync.dma_start(out=qt_sb[c][:, 512:2048],
                              in_=qT[c * 128:(c + 1) * 128, 512:2048])
            nc.sync.dma_start(out=kt_sb[c][:, 512:1024],
                              in_=kT[c * 128:(c + 1) * 128, 512:1024])
        else:
            nc.sync.dma_start(out=qt_sb[c][:, :],
                              in_=qT[c * 128:(c + 1) * 128, :])
            nc.sync.dma_start(out=kt_sb[c][:, :],
                              in_=kT[c * 128:(c + 1) * 128, :])
    nc.sync.dma_start(out=sel_sb[:, :], in_=sel)

    # ---- GEMM1 + local softmax stats per q tile ----------------
    RAMP = 4  # first tiles run chunk-major so each arriving chunk feeds 8 MMs
    # staircase: tile qi consumes chunk s-qi at step s, so tile completions
    # stagger and the softmax consumers drain while later tiles finish
    ramp_ps = [PS.tile([128, S2L], F32, tag="ps", name=f"s{qi}")
               for qi in range(RAMP)]
    for s in range(NDC + RAMP - 1):
        for qi in range(RAMP):
            dc = s - qi
            if not 0 <= dc < NDC:
                continue
            for kb in range(NKB):
                nc.tensor.matmul(
                    ramp_ps[qi][:, kb * 512:(kb + 1) * 512],
                    lhsT=qt_sb[dc][:, qi * 128:(qi + 1) * 128],
                    rhs=kt_sb[dc][:, kb * 512:(kb + 1) * 512],
                    start=(dc == 0),
                    stop=(dc == NDC - 1),
                )
    for qi in range(NQT):
        if qi < RAMP:
            ps = ramp_ps[qi]
        else:
            ps = PS.tile([128, S2L], F32, tag="ps", name=f"s{qi}")
            for dc in range(NDC):
                for kb in range(NKB):
                    nc.tensor.matmul(
                        ps[:, kb * 512:(kb + 1) * 512],
                        lhsT=qt_sb[dc][:, qi * 128:(qi + 1) * 128],
                        rhs=kt_sb[dc][:, kb * 512:(kb + 1) * 512],
                        start=(dc == 0),
                        stop=(dc == NDC - 1),
                    )
        nc.vector.tensor_reduce(
            out=negm[:, qi:qi + 1], in_=ps[:, :],
            axis=mybir.AxisListType.X, op=mybir.AluOpType.max, negate=True)
        # E = exp(S - m_loc) (fp16), Z_loc = row-sum(E) (f32)
        nc.scalar.activation(
            out=e_sb[qi][:, :], in_=ps[:, :],
            func=mybir.ActivationFunctionType.Exp,
            bias=negm[:, qi:qi + 1], scale=1.0,
            accum_out=zloc[:, qi:qi + 1])

        # overlap: V tile load + add while GEMM1 runs
        enc_t = ST.tile([128, D], FP16, tag="enc", name=f"enc{qi}")
        res_t = ST.tile([128, D], FP16, tag="res", name=f"res{qi}")
        nc.sync.dma_start(out=enc_t[:, :],
                          in_=enc[qi * 128:(qi + 1) * 128, :])
        nc.sync.dma_start(out=res_t[:, :],
                          in_=res[qi * 128:(qi + 1) * 128, :])
        nc.vector.tensor_add(out=v_sb[qi][:, :], in0=enc_t[:, :],
                             in1=res_t[:, :])

        if qi + 1 in SPLITS:
            # mid-GEMM1 stats exchange: latency hides under remaining
            # GEMM1 tiles
            lo = ([0] + list(SPLITS))[SPLITS.index(qi + 1)]
            _emit_stats_exchange(nc, P, DR, sel_sb, negm, zloc, cs, lo,
                                 qi + 1, f"x{qi + 1}", use_collective)
            for qj in range(lo, qi + 1):
                nc.vector.tensor_scalar_mul(
                    out=v_sb[qj][:, :], in0=v_sb[qj][:, :],
                    scalar1=cs[:, qj:qj + 1])

    # final stats exchange: latency hides under GEMM2's earlier-phase
    # accumulation below
    _emit_stats_exchange(nc, P, DR, sel_sb, negm, zloc, cs, SPLITS[-1], NQT,
                         "z", use_collective)
    for qj in range(SPLITS[-1], NQT):
        nc.vector.tensor_scalar_mul(
            out=v_sb[qj][:, :], in0=v_sb[qj][:, :],
            scalar1=cs[:, qj:qj + 1])

    # ---- GEMM2: out[k, d] = sum_q E[q, k] * V'[q, d] ------------
    # ki-sets of 4/3/1 psum tiles; each [128, 1024] tile holds two 512-wide
    # accumulation groups, so up to 8 groups are open at once. Groups
    # accumulate q tiles phase by phase following SPLITS, so each phase's
    # V' tiles are ready (stats exchanged) before its matmuls issue; the
    # small final set keeps the kernel tail short.
    phases = [0] + list(SPLITS) + [NQT]
    ki_sets = [range(0, 4), range(4, 7), range(7, 8)]
    for kis in ki_sets:
        psg = {}
        for pi in range(len(phases) - 1):
            last_phase = pi == len(phases) - 2
            for ki in kis:
                if pi == 0:
                    psg[ki] = PS.tile([128, S2L], F32, tag="ps",
                                      name=f"o{ki}")
                ps2 = psg[ki]
                final_ki = last_phase and ki == kis[-1] and kis is ki_sets[-1]
                if final_ki:
                    # db-major so the db=0 bank finishes early and its
                    # copy-out overlaps the very last matmuls
                    for db in range(2):
                        for qi in range(phases[pi], phases[pi + 1]):
                            nc.tensor.matmul(
                                ps2[:, db * 512:(db + 1) * 512],
                                lhsT=e_sb[qi][:, ki * 128:(ki + 1) * 128],
                                rhs=v_sb[qi][:, db * 512:(db + 1) * 512],
                                start=(qi == 0),
                                stop=(qi == NQT - 1),
                            )
                else:
                    for qi in range(phases[pi], phases[pi + 1]):
                        for db in range(2):
                            nc.tensor.matmul(
                                ps2[:, db * 512:(db + 1) * 512],
                                lhsT=e_sb[qi][:, ki * 128:(ki + 1) * 128],
                                rhs=v_sb[qi][:, db * 512:(db + 1) * 512],
                                start=(qi == 0),
                                stop=(qi == NQT - 1),
                            )
                if last_phase:
                    # copy+store this ki while later kis' matmuls still run;
                    # for the final ki, put the two copies on different
                    # engines so the tail is not DVE-serialized
                    ot = OST.tile([128, D], F32, tag="ot", name=f"ot{ki}")
                    for db in range(2):
                        if final_ki and db == 1:
                            nc.scalar.copy(
                                out=ot[:, db * 512:(db + 1) * 512],
                                in_=ps2[:, db * 512:(db + 1) * 512])
                        else:
                            nc.vector.tensor_copy(
                                out=ot[:, db * 512:(db + 1) * 512],
                                in_=ps2[:, db * 512:(db + 1) * 512])
                        nc.sync.dma_start(
                            out=out[ki * 128:(ki + 1) * 128,
                                    db * 512:(db + 1) * 512],
                            in_=ot[:, db * 512:(db + 1) * 512])


def _build_kernel(nc, qT, kT, enc, res, sel, out, reps=1,
                  use_collective=True):
    tc = tile.TileContext(nc)
    with tc:
        with (
            tc.tile_pool(name="persist", bufs=1) as P,
            tc.tile_pool(name="stage", bufs=4) as ST,
            tc.tile_pool(name="psum", bufs=4, space="PSUM") as PS,
            tc.tile_pool(name="outst", bufs=4) as OST,
            tc.tile_pool(name="dram", bufs=1, space="DRAM") as DR,
        ):
            pools = (P, ST, PS, OST, DR)
            for _ in range(reps):
                _emit_body(nc, tc, pools, qT, kT, enc, res, sel, out,
                           use_collective)
    return nc


def build(reps=1, use_collective=True):
    nc = bacc.Bacc("TRN2", target_bir_lowering=False, debug=False,
                   num_devices=N_CORES)
    qT = nc.dram_tensor("qT", [D, S], FP16, kind="ExternalInput").ap()
    kT = nc.dram_tensor("kT", [D, S2L], FP16, kind="ExternalInput").ap()
    enc = nc.dram_tensor("enc", [S, D], FP16, kind="ExternalInput").ap()
    res = nc.dram_tensor("res", [S, D], FP16, kind="ExternalInput").ap()
    sel = nc.dram_tensor("sel", [128, N_CORES], F32,
                         kind="ExternalInput").ap()
    out = nc.dram_tensor("out", [S2L, D], F32, kind="ExternalOutput").ap()
    _build_kernel(nc, qT, kT, enc, res, sel, out, reps=reps,
                  use_collective=use_collective)
    nc.compile()
    return nc


def make_in_maps(enc_outputs, atten_outputs, enc_residual):
    enc_outputs = np.asarray(enc_outputs, dtype=np.float32)
    atten_outputs = np.asarray(atten_outputs, dtype=np.float32)
    enc_residual = np.asarray(enc_residual, dtype=np.float32)
    enc16 = enc_outputs.astype(np.float16)
    att16 = atten_outputs.astype(np.float16)
    res16 = enc_residual.astype(np.float16)
    in_maps = []
    for core in range(N_CORES):
        b, half = core // 2, core % 2
        sel = np.zeros((128, N_CORES), np.float32)
        sel[:, core ^ 1] = 1.0
        in_maps.append({
            "qT": np.ascontiguousarray(enc16[b].T),
            "kT": np.ascontiguousarray(att16[b, half * S2L:(half + 1) * S2L, :].T),
            "enc": enc16[b],
            "res": res16[b],
            "sel": sel,
        })
    return in_maps


def assemble(results):
    out = np.empty((B, S, D), np.float32)
    for core in range(N_CORES):
        b, half = core // 2, core % 2
        out[b, half * S2L:(half + 1) * S2L, :] = results[core]["out"]
    return out


_NC = None


def kernel(enc_outputs, atten_outputs, enc_residual):
    global _NC
    if _NC is None:
        _NC = build()
    in_maps = make_in_maps(enc_outputs, atten_outputs, enc_residual)
    last_err = None
    for _attempt in range(3):
        try:
            res = run_bass_kernel_spmd(_NC, in_maps,
                                       core_ids=list(range(N_CORES)))
            return assemble(res.results)
        except Exception as e:  # transient device/tunnel errors -- retry
            last_err = e
    raise last_err


# revision 24
# speedup vs baseline: 1.0477x; 1.0477x over previous
"""Distributed Trainium2 Bass kernel for nn_Attention_87368224735328.

reference:
    score = einsum("bqd,bkd->bqk", enc_outputs, atten_outputs)   # [B,S1,S2]
    alignment = softmax(score, axis=-1)                          # over S2
    out = einsum("bqk,bqd->bkd", alignment, enc_outputs + enc_residual)

Sharding: 8 cores = (batch b in 0..3) x (S2-half in 0..1). Each core computes
its local [S1, S2/2] score block, local softmax row-stats (max / sum-exp over
its S2 half), exchanges the tiny [S1] stats with its partner core, and runs
the second GEMM fully locally (contraction over S1 is complete on every
core). Output shard: [S2/2, D] -> out[b, half].

Stats exchange: one-chip 8-core AllGather (the 2-rank-group collective path
measures ~16x slower than the 8-core path on this stack), with the partner's
slice extracted rank-agnostically via a host-provided one-hot mask so the
SPMD graph stays identical across cores. The exchange is split in three
(q-tile boundaries SPLITS) so every AllGather's latency hides under
TensorEngine work: the early ones under remaining GEMM1 tiles, the last
under GEMM2's earlier-phase accumulation (8 concurrently-open PSUM groups).

Precision: fp16 operands on the TensorEngine (full rate, ~16x finer mantissa
than bf16 -- needed because the scores have std ~32 so softmax is nearly
one-hot and bf16 score error flips argmaxes). Accumulation is f32 in PSUM,
stats/softmax math in f32. Measured end-to-end rel err vs f32 reference ~1.6e-3.
"""

import numpy as np

from concourse import bacc, mybir, tile
from concourse.bass_utils import run_bass_kernel_spmd

B, S, D = 4, 2048, 1024
S2L = S // 2          # local S2 columns per core
NQT = S // 128        # 16 q tiles (S1)
NDC = D // 128        # 8 contraction chunks for GEMM1
NKB = S2L // 512      # 2 PSUM blocks of 512 for GEMM1
NKT = S2L // 128      # 8 output k tiles for GEMM2
SPLITS = (10, 14)     # stats-exchange boundaries (in q tiles)
FP16 = mybir.dt.float16
F32 = mybir.dt.float32
N_CORES = 8
RG8 = [[0, 1, 2, 3, 4, 5, 6, 7]]


def _emit_stats_exchange(nc, P, DR, sel_sb, negm, zloc, cs, lo, hi, tag,
                         use_collective):
    """AllGather all cores' (-m, z) for q tiles [lo, hi), pick the partner's
    slice with the one-hot mask, and write cs[:, lo:hi]."""
    n = hi - lo
    stats_in = DR.tile([128, 2 * n], F32, name=f"stats_in{tag}")
    stats_out = DR.tile([N_CORES, 128, 2 * n], F32, name=f"stats_out{tag}")
    # scalar HWDGE queue: the sync queue is backlogged with bulk streaming
    nc.scalar.dma_start(out=stats_in[:, 0:n], in_=negm[:, lo:hi])
    nc.scalar.dma_start(out=stats_in[:, n:2 * n], in_=zloc[:, lo:hi])
    if use_collective:
        nc.gpsimd.collective_compute(
            "AllGather", mybir.AluOpType.bypass,
            replica_groups=RG8,
            ins=[stats_in[:, :].opt()],
            outs=[stats_out[:, :, :].opt()],
        )
    else:  # debug/sim variant: pretend every rank has our stats
        for r in range(N_CORES):
            nc.scalar.dma_start(out=stats_out[r], in_=stats_in[:, :])
    gath = P.tile([128, N_CORES, 2 * n], F32, tag=f"gath{tag}",
                  name=f"gath{tag}")
    nc.scalar.dma_start(out=gath[:, :, :],
                        in_=stats_out[:, :, :].rearrange("r p c -> p r c"))

    # partner slice = sum_r sel[r] * gath[r]  (sel is one-hot at partner)
    acc = P.tile([128, 2 * n], F32, tag=f"acc{tag}", name=f"acc{tag}")
    nc.vector.tensor_scalar_mul(out=acc[:, :], in0=gath[:, 0, :],
                                scalar1=sel_sb[:, 0:1])
    for r in range(1, N_CORES):
        nc.vector.scalar_tensor_tensor(
            out=acc[:, :], in0=gath[:, r, :], scalar=sel_sb[:, r:r + 1],
            in1=acc[:, :], op0=mybir.AluOpType.mult, op1=mybir.AluOpType.add)

    # all in negated-max terms: ng = -m_glob = min(negm0, negm1);
    # t_i = exp(ng - negm_i) = exp(m_i - m_glob)
    n0, z0 = negm[:, lo:hi], zloc[:, lo:hi]
    n1, z1 = acc[:, 0:n], acc[:, n:2 * n]
    ng = P.tile([128, n], F32, tag=f"ng{tag}", name=f"ng{tag}")
    t0 = P.tile([128, n], F32, tag=f"t0{tag}", name=f"t0{tag}")
    t1 = P.tile([128, n], F32, tag=f"t1{tag}", name=f"t1{tag}")
    zg = P.tile([128, n], F32, tag=f"zg{tag}", name=f"zg{tag}")
    rz = P.tile([128, n], F32, tag=f"rz{tag}", name=f"rz{tag}")
    nc.vector.tensor_tensor(out=ng[:, :], in0=n0, in1=n1,
                            op=mybir.AluOpType.min)
    nc.vector.tensor_sub(out=t0[:, :], in0=ng[:, :], in1=n0)
    nc.vector.tensor_sub(out=t1[:, :], in0=ng[:, :], in1=n1)
    nc.scalar.activation(out=t0[:, :], in_=t0[:, :],
                         func=mybir.ActivationFunctionType.Exp)
    nc.scalar.activation(out=t1[:, :], in_=t1[:, :],
                         func=mybir.ActivationFunctionType.Exp)
    nc.vector.tensor_mul(out=zg[:, :], in0=t0[:, :], in1=z0)
    nc.vector.tensor_mul(out=t1[:, :], in0=t1[:, :], in1=z1)
    nc.vector.tensor_add(out=zg[:, :], in0=zg[:, :], in1=t1[:, :])
    # c = exp(m_loc - m_glob) / Z_glob = t0 / Z_glob
    nc.vector.reciprocal(out=rz[:, :], in_=zg[:, :])
    nc.vector.tensor_mul(out=cs[:, lo:hi], in0=t0[:, :], in1=rz[:, :])


def _emit_body(nc, tc, pools, qT, kT, enc, res, sel, out, use_collective):
    P, ST, PS, OST, DR = pools

    # ---- persistent SBUF tensors -------------------------------
    qt_sb = [P.tile([128, S], FP16, tag=f"qt{c}", name=f"qt{c}")
             for c in range(NDC)]
    kt_sb = [P.tile([128, S2L], FP16, tag=f"kt{c}", name=f"kt{c}")
             for c in range(NDC)]
    v_sb = [P.tile([128, D], FP16, tag=f"v{i}", name=f"v{i}")
            for i in range(NQT)]
    e_sb = [P.tile([128, S2L], FP16, tag=f"e{i}", name=f"e{i}")
            for i in range(NQT)]
    negm = P.tile([128, NQT], F32, tag="negm", name="negm")
    zloc = P.tile([128, NQT], F32, tag="zloc", name="zloc")
    cs = P.tile([128, NQT], F32, tag="cs", name="cs")
    sel_sb = P.tile([128, N_CORES], F32, tag="sel", name="sel_sb")

    # ---- load GEMM1 operands (d on partitions, pre-transposed) --
    # Two HWDGE queues in parallel: kt chunks issue from the (ramp-idle)
    # scalar engine, qt from sync. qt is streamed in two column waves so
    # the ramp tiles' columns [0:512) all land first.
    for c in range(NDC):
        nc.scalar.dma_start(out=kt_sb[c][:, :],
                            in_=kT[c * 128:(c + 1) * 128, :])
        nc.sync.dma_start(out=qt_sb[c][:, 0:512],
                          in_=qT[c * 128:(c + 1) * 128, 0:512])
    for c in range(NDC):
        nc.sync.dma_start(out=qt_sb[c][:, 512:2048],
                          in_=qT[c * 128:(c + 1) * 128, 512:2048])
    nc.sync.dma_start(out=sel_sb[:, :], in_=sel)

    # ---- GEMM1 + local softmax stats per q tile ----------------
    RAMP = 4  # first tiles run chunk-major so each arriving chunk feeds 8 MMs
    # staircase: tile qi consumes chunk s-qi at step s, so tile completions
    # stagger and the softmax consumers drain while later tiles finish
    ramp_ps = [PS.tile([128, S2L], F32, tag="ps", name=f"s{qi}")
               for qi in range(RAMP)]
    for s in range(NDC + RAMP - 1):
        for qi in range(RAMP):
            dc = s - qi
            if not 0 <= dc < NDC:
                continue
            for kb in range(NKB):
                nc.tensor.matmul(
                    ramp_ps[qi][:, kb * 512:(kb + 1) * 512],
                    lhsT=qt_sb[dc][:, qi * 128:(qi + 1) * 128],
                    rhs=kt_sb[dc][:, kb * 512:(kb + 1) * 512],
                    start=(dc == 0),
                    stop=(dc == NDC - 1),
                )
    for qi in range(NQT):
        if qi < RAMP:
            ps = ramp_ps[qi]
        else:
            ps = PS.tile([128, S2L], F32, tag="ps", name=f"s{qi}")
            for dc in range(NDC):
                for kb in range(NKB):
                    nc.tensor.matmul(
                        ps[:, kb * 512:(kb + 1) * 512],
                        lhsT=qt_sb[dc][:, qi * 128:(qi + 1) * 128],
                        rhs=kt_sb[dc][:, kb * 512:(kb + 1) * 512],
                        start=(dc == 0),
                        stop=(dc == NDC - 1),
                    )
        nc.vector.tensor_reduce(
            out=negm[:, qi:qi + 1], in_=ps[:, :],
            axis=mybir.AxisListType.X, op=mybir.AluOpType.max, negate=True)
        # E = exp(S - m_loc) (fp16), Z_loc = row-sum(E) (f32)
        nc.scalar.activation(
            out=e_sb[qi][:, :], in_=ps[:, :],
            func=mybir.ActivationFunctionType.Exp,
            bias=negm[:, qi:qi + 1], scale=1.0,
            accum_out=zloc[:, qi:qi + 1])

        # overlap: V tile load + add while GEMM1 runs
        enc_t = ST.tile([128, D], FP16, tag="enc", name=f"enc{qi}")
        res_t = ST.tile([128, D], FP16, tag="res", name=f"res{qi}")
        nc.sync.dma_start(out=enc_t[:, :],
                          in_=enc[qi * 128:(qi + 1) * 128, :])
        nc.sync.dma_start(out=res_t[:, :],
                          in_=res[qi * 128:(qi + 1) * 128, :])
        nc.vector.tensor_add(out=v_sb[qi][:, :], in0=enc_t[:, :],
                             in1=res_t[:, :])

        if qi + 1 in SPLITS:
            # mid-GEMM1 stats exchange: latency hides under remaining
            # GEMM1 tiles
            lo = ([0] + list(SPLITS))[SPLITS.index(qi + 1)]
            _emit_stats_exchange(nc, P, DR, sel_sb, negm, zloc, cs, lo,
                                 qi + 1, f"x{qi + 1}", use_collective)
            for qj in range(lo, qi + 1):
                nc.vector.tensor_scalar_mul(
                    out=v_sb[qj][:, :], in0=v_sb[qj][:, :],
                    scalar1=cs[:, qj:qj + 1])

    # final stats exchange: latency hides under GEMM2's earlier-phase
    # accumulation below
    _emit_stats_exchange(nc, P, DR, sel_sb, negm, zloc, cs, SPLITS[-1], NQT,
                         "z", use_collective)
    for qj in range(SPLITS[-1], NQT):
        nc.vector.tensor_scalar_mul(
            out=v_sb[qj][:, :], in0=v_sb[qj][:, :],
            scalar1=cs[:, qj:qj + 1])

    # ---- GEMM2: out[k, d] = sum_q E[q, k] * V'[q, d] ------------
    # ki-sets of 4/3/1 psum tiles; each [128, 1024] tile holds two 512-wide
    # accumulation groups, so up to 8 groups are open at once. Groups
    # accumulate q tiles phase by phase following SPLITS, so each phase's
    # V' tiles are ready (stats exchanged) before its matmuls issue; the
    # small final set keeps the kernel tail short.
    phases = [0] + list(SPLITS) + [NQT]
    ki_sets = [range(0, 4), range(4, 7), range(7, 8)]
    for kis in ki_sets:
        psg = {}
        for pi in range(len(phases) - 1):
            last_phase = pi == len(phases) - 2
            for ki in kis:
                if pi == 0:
                    psg[ki] = PS.tile([128, S2L], F32, tag="ps",
                                      name=f"o{ki}")
                ps2 = psg[ki]
                final_ki = last_phase and ki == kis[-1] and kis is ki_sets[-1]
                if final_ki:
                    # db-major so the db=0 bank finishes early and its
                    # copy-out overlaps the very last matmuls
                    for db in range(2):
                        for qi in range(phases[pi], phases[pi + 1]):
                            nc.tensor.matmul(
                                ps2[:, db * 512:(db + 1) * 512],
                                lhsT=e_sb[qi][:, ki * 128:(ki + 1) * 128],
                                rhs=v_sb[qi][:, db * 512:(db + 1) * 512],
                                start=(qi == 0),
                                stop=(qi == NQT - 1),
                            )
                else:
                    for qi in range(phases[pi], phases[pi + 1]):
                        for db in range(2):
                            nc.tensor.matmul(
                                ps2[:, db * 512:(db + 1) * 512],
                                lhsT=e_sb[qi][:, ki * 128:(ki + 1) * 128],
                                rhs=v_sb[qi][:, db * 512:(db + 1) * 512],
                                start=(qi == 0),
                                stop=(qi == NQT - 1),
                            )
                if last_phase:
                    # copy+store this ki while later kis' matmuls still run;
                    # for the final ki, put the two copies on different
                    # engines so the tail is not DVE-serialized
                    ot = OST.tile([128, D], F32, tag="ot", name=f"ot{ki}")
                    for db in range(2):
                        if final_ki and db == 1:
                            nc.scalar.copy(
                                out=ot[:, db * 512:(db + 1) * 512],
                                in_=ps2[:, db * 512:(db + 1) * 512])
                            nc.scalar.dma_start(
                                out=out[ki * 128:(ki + 1) * 128,
                                        db * 512:(db + 1) * 512],
                                in_=ot[:, db * 512:(db + 1) * 512])
                        else:
                            nc.vector.tensor_copy(
                                out=ot[:, db * 512:(db + 1) * 512],
                                in_=ps2[:, db * 512:(db + 1) * 512])
                            nc.sync.dma_start(
                                out=out[ki * 128:(ki + 1) * 128,
                                        db * 512:(db + 1) * 512],
                                in_=ot[:, db * 512:(db + 1) * 512])


def _build_kernel(nc, qT, kT, enc, res, sel, out, reps=1,
                  use_collective=True):
    tc = tile.TileContext(nc)
    with tc:
        with (
            tc.tile_pool(name="persist", bufs=1) as P,
            tc.tile_pool(name="stage", bufs=4) as ST,
            tc.tile_pool(name="psum", bufs=4, space="PSUM") as PS,
            tc.tile_pool(name="outst", bufs=4) as OST,
            tc.tile_pool(name="dram", bufs=1, space="DRAM") as DR,
        ):
            pools = (P, ST, PS, OST, DR)
            for _ in range(reps):
                _emit_body(nc, tc, pools, qT, kT, enc, res, sel, out,
                           use_collective)
    return nc


def build(reps=1, use_collective=True):
    nc = bacc.Bacc("TRN2", target_bir_lowering=False, debug=False,
                   num_devices=N_CORES)
    qT = nc.dram_tensor("qT", [D, S], FP16, kind="ExternalInput").ap()
    kT = nc.dram_tensor("kT", [D, S2L], FP16, kind="ExternalInput").ap()
    enc = nc.dram_tensor("enc", [S, D], FP16, kind="ExternalInput").ap()
    res = nc.dram_tensor("res", [S, D], FP16, kind="ExternalInput").ap()
    sel = nc.dram_tensor("sel", [128, N_CORES], F32,
                         kind="ExternalInput").ap()
    out = nc.dram_tensor("out", [S2L, D], F32, kind="ExternalOutput").ap()
    _build_kernel(nc, qT, kT, enc, res, sel, out, reps=reps,
                  use_collective=use_collective)
    nc.compile()
    return nc


def make_in_maps(enc_outputs, atten_outputs, enc_residual):
    enc_outputs = np.asarray(enc_outputs, dtype=np.float32)
    atten_outputs = np.asarray(atten_outputs, dtype=np.float32)
    enc_residual = np.asarray(enc_residual, dtype=np.float32)
    enc16 = enc_outputs.astype(np.float16)
    att16 = atten_outputs.astype(np.float16)
    res16 = enc_residual.astype(np.float16)
    in_maps = []
    for core in range(N_CORES):
        b, half = core // 2, core % 2
        sel = np.zeros((128, N_CORES), np.float32)
        sel[:, core ^ 1] = 1.0
        in_maps.append({
            "qT": np.ascontiguousarray(enc16[b].T),
            "kT": np.ascontiguousarray(att16[b, half * S2L:(half + 1) * S2L, :].T),
            "enc": enc16[b],
            "res": res16[b],
            "sel": sel,
        })
    return in_maps


def assemble(results):
    out = np.empty((B, S, D), np.float32)
    for core in range(N_CORES):
        b, half = core // 2, core % 2
        out[b, half * S2L:(half + 1) * S2L, :] = results[core]["out"]
    return out


_NC = None


def kernel(enc_outputs, atten_outputs, enc_residual):
    global _NC
    if _NC is None:
        _NC = build()
    in_maps = make_in_maps(enc_outputs, atten_outputs, enc_residual)
    last_err = None
    for _attempt in range(3):
        try:
            res = run_bass_kernel_spmd(_NC, in_maps,
                                       core_ids=list(range(N_CORES)))
            return assemble(res.results)
        except Exception as e:  # transient device/tunnel errors -- retry
            last_err = e
    raise last_err
